# revision 50
# baseline (speedup 1.0000x reference)
"""Trainium2 Bass kernel for LocalDownsampleFlexAttn (24-head attention with
pooled-KV augmentation), head-parallel across 8 NeuronCores.

Sharding: each core owns 3 of the 24 heads. Per core:
  - QKV projections for its 3 heads (column-sliced Wq/Wk/Wv)
  - KV downsampling (4x4 spatial pooling of the 1024 image tokens -> 64)
  - attention over 1536+64 keys
  - partial output projection (row-sliced Wo); host sums the 8 partials + bo.

Layout strategy (v3):
  - x is transposed + cast to bf16 on the host; all weights bf16; output f16.
  - scores are computed transposed ([key, query]) into a pair of ping-pong
    [128,3,512] PSUM tiles, one per key-tile, so scores(c+1) never waits on
    exp(c): the PE and ACT pipeline two-deep and phase C runs at the ACT
    (exp) rate of ~1.5us per key-tile instead of the serialized ~2.4us.
  - PV runs as three sequential per-query-group passes of 13 chained
    matmuls into a single PSUM bank each (2-slot pool), interleaved into
    the NEXT head's score periods (and into phase D for the last head) as
    PE filler; each pass ends with the normalizing multiply (psum * 1/sums)
    that writes attnT directly — no unnormalized-attn staging buffer.
  - softmax sums: DVE bf16 tree-add over probsT c-tiles 0..11; the pooled
    tile joins via a second accumulating M=1 matmul; the [1,1536] sum row
    is DMA-bounced to [128,12] so the DVE reciprocal runs on 128 lanes,
    PE-transposed back, and broadcast-read from DRAM into [128,1536].
  - phase D pre-runs the first two chunks' h0/h1 accumulation during h2's
    pv/normalization, drains the last chunks in 512-col slices, and picks
    PSUM slots that avoid the banks h2's norm chain still occupies.
"""

import numpy as np
from contextlib import ExitStack

# ---- problem constants (hardcoded per harness contract) ----
S = 1536          # sequence length
DM = 3072         # model dim
NH = 24           # total heads
HD = 128          # head dim
NCORES = 8
HPC = NH // NCORES   # heads per core = 3
CW = HPC * HD        # per-core slice width = 384
TXT = 512
IMG = 1024        # image tokens (32x32)
F = 4             # pooling factor
PK = (IMG // (F * F))  # pooled keys = 64
KALL = S + PK     # 1600 keys
NKT = DM // 128   # 24 model-dim k-tiles
NTT = S // 128    # 12 token tiles
NIT = IMG // 128  # 8 image-token tiles
NKC = (KALL + 127) // 128   # 13 key tiles (last has 64)
NRING = 6         # scores PSUM ring slots
ASCALE = float((1.0 / HD) ** 0.5)

_CACHE = {}


def _build_program(debug_taps=False):
    import concourse.bass as bass
    import concourse.bacc as bacc
    import concourse.tile as tile
    from concourse import mybir
    from concourse.masks import make_identity

    f32 = mybir.dt.float32
    f16 = mybir.dt.float16
    bf16 = mybir.dt.bfloat16
    AF = mybir.ActivationFunctionType
    AX = mybir.AxisListType

    nc = bacc.Bacc(
        "TRN2",
        target_bir_lowering=False,
        debug=False,
        enable_asserts=False,
        num_devices=NCORES,
    )

    xt_d = nc.dram_tensor("xt", [DM, S], bf16, kind="ExternalInput").ap()
    wq_d = nc.dram_tensor("wq", [DM, CW], bf16, kind="ExternalInput").ap()
    wk_d = nc.dram_tensor("wk", [DM, CW], bf16, kind="ExternalInput").ap()
    wv_d = nc.dram_tensor("wv", [DM, CW], bf16, kind="ExternalInput").ap()
    bq_d = nc.dram_tensor("bq", [CW], f32, kind="ExternalInput").ap()
    bk_d = nc.dram_tensor("bk", [CW], f32, kind="ExternalInput").ap()
    bv_d = nc.dram_tensor("bv", [CW], bf16, kind="ExternalInput").ap()
    wo_d = nc.dram_tensor("wo", [CW, DM], bf16, kind="ExternalInput").ap()
    pmat_d = nc.dram_tensor("pmat", [IMG, PK], bf16, kind="ExternalInput").ap()
    wfull_d = nc.dram_tensor("wfull", [IMG], f32, kind="ExternalInput").ap()
    out_d = nc.dram_tensor("out", [S, DM], f16, kind="ExternalOutput").ap()

    taps = {}
    if debug_taps:
        taps["qT"] = nc.dram_tensor("dbg_qT", [128, HPC, S], bf16, kind="ExternalOutput").ap()
        taps["kT"] = nc.dram_tensor("dbg_kT", [128, HPC, NKC * 128], bf16, kind="ExternalOutput").ap()
        taps["vA"] = nc.dram_tensor("dbg_vA", [128, HPC, NKC, HD], bf16, kind="ExternalOutput").ap()
        taps["sums0"] = nc.dram_tensor("dbg_sums0", [128, S], bf16, kind="ExternalOutput").ap()
        taps["attnT"] = nc.dram_tensor("dbg_attnT", [128, HPC, S], bf16, kind="ExternalOutput").ap()

    # engine alternator for PSUM->SBUF copies
    _flip = [0]

    def copy_alt(dst, src):
        _flip[0] ^= 1
        if _flip[0]:
            nc.vector.tensor_copy(dst, src)
        else:
            nc.scalar.copy(dst, src)

    with tile.TileContext(nc) as tc, ExitStack() as ctx:
        persist = ctx.enter_context(tc.tile_pool(name="persist", bufs=1))

        # per-head per-partition biases: b[p, h] = bias[h*128 + p]
        bq_sb = persist.tile([128, HPC], f32)
        bk_sb = persist.tile([128, HPC], f32)
        nc.sync.dma_start(
            out=bq_sb, in_=bass.AP(tensor=bq_d.tensor, offset=0, ap=[[1, 128], [128, HPC]])
        )
        nc.sync.dma_start(
            out=bk_sb, in_=bass.AP(tensor=bk_d.tensor, offset=0, ap=[[1, 128], [128, HPC]])
        )
        # v-bias as a [1, CW] row for the K=1 outer-product trick
        bvrow = persist.tile([1, CW], bf16)
        nc.sync.dma_start(out=bvrow, in_=bv_d[None, :])
        ones_row = persist.tile([1, 128], bf16)
        nc.vector.memset(ones_row, 1.0)

        # persistent activations
        qT = persist.tile([128, HPC, S], bf16)          # q^T per head [d, tok]
        kT = persist.tile([128, HPC, NKC * 128], bf16)  # k_all^T per head [d, key]
        vA = persist.tile([128, HPC, NKC, HD], bf16)    # v_all per head [key, kt, d]
        attnT = persist.tile([128, HPC, S], bf16)       # attn^T [d(by head), tok]

        # ---------------- Phase B: QKV projections ----------------
        es_b = ctx.enter_context(ExitStack())
        pw = es_b.enter_context(tc.tile_pool(name="pw", bufs=1))
        pxt = es_b.enter_context(tc.tile_pool(name="pxt", bufs=1))
        # per-kt weight + x^T tiles; weights load in 4-kt groups (384KB per
        # DMA) interleaved with the x tiles so q(h0) and k(h0) both stream
        WG = 4
        NG = NKT // WG

        def w_group(w_d, g, nm):
            t = pw.tile([128, WG, CW], bf16, name=f"{nm}{g}", tag=f"{nm}{g}")
            nc.sync.dma_start(
                out=t,
                in_=bass.AP(tensor=w_d.tensor, offset=g * WG * 128 * CW,
                            ap=[[CW, 128], [128 * CW, WG], [1, CW]]),
            )
            return t

        xts = []
        wq_t = []
        wk_t = []
        for kt in range(NKT):
            if kt % WG == 0:
                g = kt // WG
                wq_t.append(w_group(wq_d, g, "wq"))
                wk_t.append(w_group(wk_d, g, "wk"))
            xtile = pxt.tile([128, S], bf16, name=f"xt{kt}", tag=f"xt{kt}")
            nc.sync.dma_start(out=xtile, in_=xt_d[kt * 128:(kt + 1) * 128, :])
            xts.append(xtile)

        # pooling inputs are consumed tens of us in — don't let them delay
        # the first x/weight tiles
        pm_bf = persist.tile([128, NIT, PK], bf16)
        nc.sync.dma_start(
            out=pm_bf,
            in_=bass.AP(tensor=pmat_d.tensor, offset=0,
                        ap=[[PK, 128], [128 * PK, NIT], [1, PK]]),
        )
        wfull_sb = persist.tile([128, IMG], f32)
        nc.sync.dma_start(
            out=wfull_sb,
            in_=bass.AP(tensor=wfull_d.tensor, offset=0, ap=[[0, 128], [1, IMG]]),
        )

        wv_t = [w_group(wv_d, g, "wv") for g in range(NG)]

        def wchunk(groups, kt, lo, hi):
            return groups[kt // WG][:, kt % WG, lo:hi]

        pBqk = es_b.enter_context(tc.tile_pool(name="pBqk", bufs=2, space="PSUM"))
        pBv = es_b.enter_context(tc.tile_pool(name="pBv", bufs=2, space="PSUM"))
        pKp = es_b.enter_context(tc.tile_pool(name="pKp", bufs=2))

        def qk_copy(h, dst, b_sb, ps):
            nc.scalar.activation(
                dst[:, h, 0:S],
                ps,
                AF.Identity,
                bias=b_sb[:, h:h + 1],
                scale=1.0,
            )

        def pooled_k(h):
            # pooled k columns (kT[:, h, 1536:1600]) via DVE weighted reduce
            tmpw = pKp.tile([128, IMG], f32, tag="tmpw")
            for R in range(8):
                nc.vector.tensor_mul(
                    tmpw[:, R * 128:(R + 1) * 128].rearrange(
                        "p (C i j) -> p C i j", C=8, i=4),
                    kT[:, h, TXT + R * 128:TXT + (R + 1) * 128].rearrange(
                        "p (i C j) -> p C i j", i=4, C=8),
                    wfull_sb[:, R * 128:(R + 1) * 128].rearrange(
                        "p (i C j) -> p C i j", i=4, C=8),
                )
            pooled = pKp.tile([128, PK], f32, tag="pooled")
            nc.vector.reduce_sum(
                pooled,
                tmpw.rearrange("p (rc ij) -> p rc ij", ij=F * F),
                axis=AX.X,
            )
            copy_alt(kT[:, h, S:S + PK], pooled)

        def v_group(tt):
            psv = pBv.tile([128, CW], f32, tag="v", name=f"psv{tt}")
            nc.tensor.matmul(psv, ones_row, bvrow, start=True, stop=False)
            return psv

        def v_step(psv, tt, kt):
            nc.tensor.matmul(
                psv,
                xts[kt][:, tt * 128:(tt + 1) * 128],
                wchunk(wv_t, kt, 0, CW),
                start=False,
                stop=(kt == NKT - 1),
            )

        def v_copy(psv, tt):
            copy_alt(
                vA[:, :, tt, :],
                psv.rearrange("p (h d) -> p h d", h=HPC),
            )

        for h in range(HPC):
            for w_t, b_sb, dst in ((wq_t, bq_sb, qT), (wk_t, bk_sb, kT)):
                ps = pBqk.tile([128, S], f32, tag="qk")
                for kt in range(NKT):
                    for c in range(3):
                        nc.tensor.matmul(
                            ps[:, c * 512:(c + 1) * 512],
                            wchunk(w_t, kt, h * 128, (h + 1) * 128),
                            xts[kt][:, c * 512:(c + 1) * 512],
                            start=(kt == 0),
                            stop=(kt == NKT - 1),
                        )
                qk_copy(h, dst, b_sb, ps)
            pooled_k(h)

        for tt in range(NTT):
            psv = v_group(tt)
            for kt in range(NKT):
                v_step(psv, tt, kt)
            v_copy(psv, tt)

        # pooled v rows (keys 1536:1600 -> tile 12, rows 0:64) — reuses the
        # pBv bank slots now that the v groups are done
        for h in range(HPC):
            psp = pBv.tile([128, CW], f32, tag="v", name=f"psp{h}")
            for it in range(NIT):
                nc.tensor.matmul(
                    psp[:PK, 0:HD],
                    pm_bf[:, it, :],
                    vA[:, h, (TXT // 128) + it, :],
                    start=(it == 0),
                    stop=(it == NIT - 1),
                )
            copy_alt(vA[:PK, h, NKC - 1, :], psp[:PK, 0:HD])

        # weights + x^T + B psum pools no longer needed
        es_b.close()

        # ---------------- Phase C: attention (ring-buffered) ----------
        # scores computed TRANSPOSED ([key, query]); the scores PSUM is a
        # 6-slot ring of [128,512] banks consumed 3 per c-tile, so slots
        # alternate {0,1,2}/{3,4,5} and scores(c) only ever waits on
        # exp(c-2): the PE and ACT pipeline 2-deep and every exp is a single
        # [cs,1536] op. pv runs as three sequential per-query-group passes
        # (13 chained matmuls into ONE psum bank each), interleaved into the
        # NEXT head's score periods. PSUM: ring 6 + pv 2 = 8 banks.
        # Softmax sums: DVE tree -> 3 M=1 matmuls into ring slots 3:6 ->
        # ACT reciprocal on the [1,1536] row -> DRAM-bounce broadcast.
        pDw = ctx.enter_context(tc.tile_pool(name="pDw", bufs=1))
        wo_sb = pDw.tile([128, HPC, DM], bf16)
        nc.sync.dma_start(
            out=wo_sb,
            in_=bass.AP(tensor=wo_d.tensor, offset=0,
                        ap=[[DM, 128], [128 * DM, HPC], [1, DM]]),
        )
        ones_col = persist.tile([128, 1], bf16)
        nc.vector.memset(ones_col, 1.0)
        ident = persist.tile([128, 128], f32)
        make_identity(nc, ident)

        es_c = ctx.enter_context(ExitStack())
        pC = es_c.enter_context(tc.tile_pool(name="pC", bufs=1))
        pCT = es_c.enter_context(tc.tile_pool(name="pCT", bufs=1))
        pCd = es_c.enter_context(tc.tile_pool(name="pCd", bufs=2, space="DRAM"))
        pCpv = ctx.enter_context(tc.tile_pool(name="pCpv", bufs=2, space="PSUM"))
        es_ring = ctx.enter_context(ExitStack())
        pCs = es_ring.enter_context(tc.tile_pool(name="pCs", bufs=2, space="PSUM"))

        state = {}  # per-head live tiles

        def emit_scores_mm(h, c, g):
            cs = 128 if c < NKC - 1 else PK
            st = state[h]
            if g == 0:
                # fresh 3-bank tile per c-tile; the pool ping-pongs between
                # two of them so scores(c+1) never waits on exp(c)
                st["psc"] = pCs.tile([128, 3, 512], f32, tag="s",
                                     name=f"psc{h}_{c}")
            nc.tensor.matmul(
                st["psc"][:cs, g, :],
                kT[:, h, c * 128:c * 128 + cs],
                qT[:, h, g * 512:(g + 1) * 512],
                start=True,
                stop=True,
            )

        def emit_exp(h, c):
            cs = 128 if c < NKC - 1 else PK
            st = state[h]
            probsT = st["probsT"]
            nc.scalar.activation(
                probsT[:cs, c, :],
                st["psc"][:cs, :, :].rearrange("p a b -> p (a b)"),
                AF.Exp, bias=0.0, scale=ASCALE,
            )

        def make_pv_pass(h, g):
            # one query-group's PV contraction: 13 matmuls chained into a
            # single psum bank, followed by the normalizing multiply that
            # writes attnT and frees the bank. Emitted a few matmuls at a
            # time inside a later head's (or phase D's) PE stream.
            st = state[h]
            ppv = pCpv.tile([128, 512], f32, tag="pv", bufs=2,
                            name=f"ppv{h}_{g}")
            probsT = st["probsT"]
            mms = []
            for c in range(NKC):
                cs = 128 if c < NKC - 1 else PK
                mms.append((ppv, probsT, h, g, c, cs))

            def emit_mm(item):
                ppv, probsT, h, g, c, cs = item
                nc.tensor.matmul(
                    ppv,
                    vA[:cs, h, c, :],
                    probsT[:cs, c, g * 512:(g + 1) * 512],
                    start=(c == 0),
                    stop=(c == NKC - 1),
                )

            def emit_ttmult():
                nc.vector.tensor_mul(
                    attnT[:, h, g * 512:(g + 1) * 512],
                    ppv,
                    st["rsb"][:, g * 512:(g + 1) * 512],
                )

            return {"mms": mms, "emit_mm": emit_mm, "ttmult": emit_ttmult}

        class PvQueue:
            def __init__(self):
                self.passes = []

            def add_head(self, h):
                for g in range(3):
                    self.passes.append(make_pv_pass(h, g))

            def step(self, n, defer=None):
                # emit up to n pv matmuls (plus the pass-final ttmult; with
                # defer, completed passes' ttmults are collected instead so
                # matmuls can flow while rsb is still in flight)
                while n > 0 and self.passes:
                    p = self.passes[0]
                    take = min(n, len(p["mms"]))
                    for item in p["mms"][:take]:
                        p["emit_mm"](item)
                    p["mms"] = p["mms"][take:]
                    n -= take
                    if not p["mms"]:
                        if defer is not None:
                            defer.append(p["ttmult"])
                        else:
                            p["ttmult"]()
                        self.passes.pop(0)

            def drain(self):
                self.step(1 << 30)

        pvq = PvQueue()

        def emit_tree(h, c):
            # softmax sums: DVE bf16 ping-pong accumulate of c-tiles 0..11;
            # the 64-key pooled tile (c=12) is folded in by the
            # partition-reduce matmuls instead.
            st = state[h]
            probsT = st["probsT"]
            if c == 1:
                st["accA"] = pC.tile([128, S], bf16, tag="sacA", bufs=2,
                                     name=f"sacA{h}")
                st["accB"] = pC.tile([128, S], bf16, tag="sacB", bufs=2,
                                     name=f"sacB{h}")
                nc.vector.tensor_add(st["accB"], probsT[:, 0, :],
                                     probsT[:, 1, :])
            elif c % 2 == 0:
                nc.vector.tensor_add(st["accA"], st["accB"],
                                     probsT[:, c, :])
            else:
                nc.vector.tensor_add(st["accB"], st["accA"],
                                     probsT[:, c, :])

        def emit_norm(h):
            # partition-reduce accA via 3 M=1 matmuls into the head's LAST
            # scores tile (already consumed by exp(h,12); reusing it keeps
            # the scores pool's ping-pong parity intact), then route the
            # [1,1536] row through DRAM into [128,12] so the DVE reciprocal
            # runs on 128 lanes (a row reciprocal is ~8 cycles/elem on one
            # lane), and broadcast the result back.
            st = state[h]
            acc = st["accB"]   # tree through c=11 ends in accB
            spsc = st["psc"]   # the c=12 tile
            for g in range(3):
                nc.tensor.matmul(
                    spsc[0:1, g, :],
                    ones_col,
                    acc[:, g * 512:(g + 1) * 512],
                    start=True,
                    stop=False,
                )
                nc.tensor.matmul(
                    spsc[0:1, g, :],
                    ones_col[:PK, :],
                    st["probsT"][:PK, NKC - 1, g * 512:(g + 1) * 512],
                    start=False,
                    stop=True,
                )
            # sums are bf16-accumulated already, so a bf16 row loses nothing;
            # 2-byte dtype enables the XBAR-transposed DMA read below (one
            # contiguous 3KB transfer instead of 128 strided descriptors)
            srow = pC.tile([1, S], bf16, tag="srow", bufs=2, name=f"srow{h}")
            nc.vector.tensor_copy(srow[:, 0:512], spsc[0:1, 0, :])
            nc.scalar.copy(srow[:, 512:1024], spsc[0:1, 1, :])
            nc.vector.tensor_copy(srow[:, 1024:1536], spsc[0:1, 2, :])
            rdram = pCd.tile([1, 16 * 128], bf16, tag="rd", bufs=2,
                             name=f"rd{h}")
            nc.sync.dma_start(out=rdram[:, 0:S], in_=srow)
            rt = pC.tile([128, 16], bf16, tag="rt", bufs=2, name=f"rt{h}")
            nc.sync.dma_start(
                out=rt,
                in_=bass.AP(tensor=rdram.tensor, offset=rdram.offset,
                            ap=[[128, 16], [1, 128]]),
                transpose=True,
            )
            rti = pC.tile([128, NTT], f32, tag="rti", bufs=2, name=f"rti{h}")
            nc.vector.reciprocal(rti, rt[:, 0:NTT])
            st["rti"] = rti

        def emit_norm_back(h):
            # PE-transpose [128,12] -> [12,128] into a corner of the spent
            # scores tile so the DRAM image is query-major and both DMAs
            # below move contiguous runs (a direct [[1,128],[128,12]] write
            # scatters 2-byte descriptors and takes ~17us). Split from the
            # front half so other matmuls can be emitted in between while
            # the reciprocal chain is in flight.
            st = state[h]
            spsc = st["psc"]
            nc.tensor.transpose(spsc[0:NTT, 2, 0:128], st["rti"], ident)
            rtt = pC.tile([NTT, 128], f32, tag="rtt", bufs=2, name=f"rtt{h}")
            nc.vector.tensor_copy(rtt, spsc[0:NTT, 2, 0:128])
            rdram2 = pCd.tile([NTT, 128], f32, tag="rd2", bufs=2,
                              name=f"rd2{h}")
            nc.sync.dma_start(out=rdram2, in_=rtt)
            rsb = pC.tile([128, S], f32, tag="rsb", bufs=2, name=f"rsb{h}")
            nc.sync.dma_start(
                out=rsb,
                in_=bass.AP(tensor=rdram2.tensor, offset=rdram2.offset,
                            ap=[[0, 128], [1, S]]),
            )
            st["rsb"] = rsb

        for h in range(HPC):
            for c in range(NKC):
                if c == 0:
                    state[h] = {
                        "probsT": pCT.tile([128, NKC, S], bf16,
                                           tag="probsT", bufs=2,
                                           name=f"probsT{h}"),
                    }
                # pv matmuls go FIRST: when a scores matmul is blocked (e.g.
                # on the previous head's norm chain touching its psum slot),
                # filler emitted after it would be trapped behind it in the
                # in-order PE queue. The first periods get a double budget —
                # they overlap the previous head's reciprocal chain.
                pvq.step(6 if c < 4 else 3)
                emit_scores_mm(h, c, 0)
                emit_scores_mm(h, c, 1)
                emit_scores_mm(h, c, 2)
                emit_exp(h, c)
                if c >= 2:
                    emit_tree(h, c - 1)
            pvq.drain()              # previous head's pv (normally a no-op)
            pvq.add_head(h)
            pvq.step(6)              # this head's first pv fills the exp(12) wait
            emit_norm(h)
            # more pv matmuls bridge the reciprocal DMA chain so neither the
            # transpose nor the next head's slot-A scores stall the PE;
            # completed passes' ttmults wait until rsb exists
            pending = []
            pvq.step(24 if h == HPC - 1 else 9, defer=pending)
            emit_norm_back(h)
            for f in pending:
                f()

        if debug_taps:
            nc.sync.dma_start(out=taps["qT"], in_=qT)
            nc.sync.dma_start(out=taps["kT"], in_=kT)
            nc.sync.dma_start(out=taps["vA"], in_=vA)

        # ring no longer needed; free its banks for the output projection
        es_ring.close()

        # ---------------- Phase D: output projection (partial) ----------------
        # h2's pv passes (queued above) interleave with the first chunks'
        # matmuls here: chunk (qt, g) only needs attnT[h2] for query group
        # qt//4, which unlocks as each pass's ttmult lands.
        with tc.tile_pool(name="pD", bufs=6) as pD, \
             tc.tile_pool(name="pDpsum", bufs=2, space="PSUM") as pDpsum:

            def d_mms(pso, qt, g, kts):
                for kt in kts:
                    for c in range(3):
                        nc.tensor.matmul(
                            pso[:, c * 512:(c + 1) * 512],
                            attnT[:, kt, qt * 128:(qt + 1) * 128],
                            wo_sb[:, kt, g * 1536 + c * 512:g * 1536 + (c + 1) * 512],
                            start=(kt == 0),
                            stop=(kt == HPC - 1),
                        )

            def d_drain(pso, qt, g, nsplit):
                # copy + DMA in nsplit column chunks so the tail pipelines
                w = 1536 // nsplit
                for i in range(nsplit):
                    outsb = pD.tile([128, w], f16, tag="outsb",
                                    name=f"outsb{qt}_{g}_{i}")
                    copy_alt(outsb, pso[:, i * w:(i + 1) * w])
                    nc.sync.dma_start(
                        out=out_d[qt * 128:(qt + 1) * 128,
                                  g * 1536 + i * w:g * 1536 + (i + 1) * w],
                        in_=outsb,
                    )

            chunks = [(qt, g) for qt in range(NTT) for g in range(2)]
            psos = {}
            # pre-run the first two chunks' h0/h1 accumulation while h2's pv
            # passes + normalization are still in flight. Slot 0 of this pool
            # aliases the scores tile that h2's norm chain still occupies, so
            # chunk 0 takes slot 1 (free as soon as exp(h2,11) retires).
            pso_slot0 = pDpsum.tile([128, 1536], f32, tag="o", name="pso_s0")
            pso_slot1 = pDpsum.tile([128, 1536], f32, tag="o", name="pso_s1")
            psos[0], psos[1] = pso_slot1, pso_slot0
            for ci in range(2):
                qt, g = chunks[ci]
                d_mms(psos[ci], qt, g, range(HPC - 1))
                pvq.step(7)
            for ci in range(len(chunks)):
                qt, g = chunks[ci]
                if ci in psos:
                    pso = psos[ci]
                    d_mms(pso, qt, g, [HPC - 1])
                else:
                    pso = pDpsum.tile([128, 1536], f32, tag="o",
                                      name=f"pso{qt}_{g}")
                    d_mms(pso, qt, g, range(HPC))
                pvq.step(7)
                d_drain(pso, qt, g, 3 if ci >= len(chunks) - 2 else 1)
            pvq.drain()

    nc.compile()
    return nc


def _get_program(debug_taps=False):
    key = ("nc", debug_taps)
    if key not in _CACHE:
        _CACHE[key] = _build_program(debug_taps=debug_taps)
    return _CACHE[key]


def _prep_in_maps(hidden_states, Wq, bq, Wk, bk, Wv, bv, Wo, spatial_weight):
    import ml_dtypes

    bf16 = ml_dtypes.bfloat16
    x = np.asarray(hidden_states, dtype=np.float32).reshape(S, DM)
    xt = np.ascontiguousarray(x.T.astype(bf16))
    Wq = np.asarray(Wq, dtype=np.float32)
    Wk = np.asarray(Wk, dtype=np.float32)
    Wv = np.asarray(Wv, dtype=np.float32)
    Wo = np.asarray(Wo, dtype=np.float32)
    bq = np.asarray(bq, dtype=np.float32)
    bk = np.asarray(bk, dtype=np.float32)
    bv = np.asarray(bv, dtype=np.float32)

    w = np.asarray(spatial_weight, dtype=np.float32).reshape(F, F)  # [i, j]
    # wfull[t] for t = 128R + 32i + 4C + j  -> broadcast w over (R, C)
    wfull = np.ascontiguousarray(
        np.broadcast_to(w[None, :, None, :], (8, F, 8, F)).reshape(IMG)
    )
    # pmat[t, R*8+C] = w[i, j] for t in block (R, C)
    pmat = np.zeros((8, F, 8, F, 8, 8), dtype=np.float32)
    for R in range(8):
        for C in range(8):
            pmat[R, :, C, :, R, C] = w
    pmat = np.ascontiguousarray(pmat.reshape(IMG, PK).astype(bf16))

    in_maps = []
    for c in range(NCORES):
        sl = slice(c * CW, (c + 1) * CW)
        in_maps.append({
            "xt": xt,
            "wq": np.ascontiguousarray(Wq[:, sl].astype(bf16)),
            "wk": np.ascontiguousarray(Wk[:, sl].astype(bf16)),
            "wv": np.ascontiguousarray(Wv[:, sl].astype(bf16)),
            "bq": np.ascontiguousarray(bq[sl]),
            "bk": np.ascontiguousarray(bk[sl]),
            "bv": np.ascontiguousarray(bv[sl].astype(bf16)),
            "wo": np.ascontiguousarray(Wo[sl, :].astype(bf16)),
            "pmat": pmat,
            "wfull": wfull,
        })
    return in_maps


def _run(inputs, trace=False, trace_kwargs=None, debug_taps=False):
    from concourse import bass_utils

    nc = _get_program(debug_taps=debug_taps)
    in_maps = _prep_in_maps(
        inputs["hidden_states"], inputs["Wq"], inputs["bq"], inputs["Wk"],
        inputs["bk"], inputs["Wv"], inputs["bv"], inputs["Wo"],
        inputs["spatial_weight"],
    )
    res = bass_utils.run_bass_kernel_spmd(
        nc, in_maps, list(range(NCORES)), trace=trace,
        **(trace_kwargs or {}),
    )
    partial = np.zeros((S, DM), dtype=np.float32)
    for r in res.results:
        partial += r["out"].astype(np.float32)
    out = partial + np.asarray(inputs["bo"], dtype=np.float32)[None, :]
    return out.reshape(1, S, DM).astype(np.float32), res


def kernel(**inputs):
    h = int(inputs.get("height", 32))
    w = int(inputs.get("width", 32))
    assert h == 32 and w == 32, (h, w)
    out, _ = _run(inputs, trace=False)
    return out
